# revision 16
# baseline (speedup 1.0000x reference)
"""Weighted-MSE loss kernel (nn_LossWithEuler) for 8 Trainium2 NeuronCores.

loss = mean(weight[b] * (inp[d,b] - label[d,b])^2)
  weight[b]  = attr_w[b] * angle_w[b]
  attr_w[b]  = sum_j (attribute[j,b]==1) * (sum(attribute_num)/attribute_num[j])
  angle_w[b] = sum_j (1 - cos(ea[j,b])) = sum_j 2*sin(ea[j,b]/2)^2

Sharding: batch axis B=131072 split across 8 cores (16384 each). Each core's
shard is host-packed to partition-major layout: partition p holds b-locals
[p*128, (p+1)*128), stored chunk-interleaved so every chunk's inp+label slice
is one contiguous run per partition (one large DMA descriptor each). The
small inputs (ea/attribute/attribute_num) are packed into a single "aux"
tensor (attribute bit-cast into the f32 stream) so they cost one DMA.

Per-core dataflow: chunked DVE subtract -> ACT square (bf16 out) -> DVE
per-sample reduce; chunk sizes taper (16x6,8x3,4,2,2 samples per partition)
so the final chunk's compute tail after the last DMA is short. The DVE
instruction order is pinned (sub(c+1) before reduce(c)) so the ACT square of
chunk c overlaps the subtract of chunk c+1. Per-partition partials are
reduced across partitions on the TensorEngine (ones-vector matmul) so each
core writes a single f32 (one DMA descriptor).
"""

import sys
import numpy as np

D = 136
B = 131072
N_CORES = 8
BS = B // N_CORES  # 16384 b's per core
P = 128            # SBUF partitions
Q = BS // P        # 128 b's per partition
# graduated chunk sizes (in b's per partition); sum must equal Q.
# Small chunks first so DVE compute starts early; small chunks last so the
# post-DMA compute tail is short; big chunks in the middle for DMA
# descriptor efficiency.
CHUNK_B = [4, 4, 8, 16, 16, 16, 16, 16, 16, 8, 4, 2, 2]
assert sum(CHUNK_B) == Q
NCHUNK = len(CHUNK_B)
TOT_F = 2 * Q * D   # f32 elements per partition in the packed data tensor
AUX_F = 3 * Q + 6 * Q + 6  # ea + attr(bitcast) + anum, f32 words / partition

_program = None


def _build_program():
    try:
        import concourse.bass as bass  # noqa: F401
    except ImportError:
        sys.path.insert(0, "/opt/trn_rl_repo")
        import concourse.bass as bass  # noqa: F401
    from concourse import bacc, mybir, tile
    from concourse.tile import add_dep_helper

    f32 = mybir.dt.float32
    i32 = mybir.dt.int32
    bf16 = mybir.dt.bfloat16
    AF = mybir.ActivationFunctionType
    OP = mybir.AluOpType
    AX = mybir.AxisListType

    nc = bacc.Bacc("TRN2", target_bir_lowering=False, debug=False,
                   num_devices=N_CORES)

    data = nc.dram_tensor("data", (P, TOT_F), f32, kind="ExternalInput")
    aux = nc.dram_tensor("aux", (P, AUX_F), f32, kind="ExternalInput")
    out = nc.dram_tensor("out", (1, 1), f32, kind="ExternalOutput")

    with tile.TileContext(nc) as tc:
        with tc.tile_pool(name="const", bufs=1) as cpool, \
             tc.tile_pool(name="data16", bufs=6) as p16, \
             tc.tile_pool(name="data8", bufs=2) as p8, \
             tc.tile_pool(name="data4", bufs=3) as p4, \
             tc.tile_pool(name="data2", bufs=2) as p2, \
             tc.tile_pool(name="psum", bufs=1, space="PSUM") as ppool:
            # ---- chunk DMAs on the SWDGE path with f32 -> bf16 cast during
            # the transfer: HBM reads stay f32, SBUF tiles are bf16, and the
            # 16-bit DVE path runs tensor_tensor at 2x. aux goes HWDGE
            # (separate queue, lands early for the weight math).
            pools = {16: p16, 8: p8, 4: p4, 2: p2}
            aux_sb = cpool.tile([P, AUX_F], f32)
            nc.sync.dma_start(aux_sb[:], aux.ap())
            dts = []
            dmas = []
            off = 0
            for c, cb in enumerate(CHUNK_B):
                f = cb * D
                t = pools[cb].tile([P, 2 * f], bf16, tag=f"d{cb}")
                dts.append(t)
                dmas.append(nc.gpsimd.dma_start(
                    t[:], data.ap()[:, off:off + 2 * f]))
                off += 2 * f
            for i in range(len(dmas) - 1):
                add_dep_helper(dmas[i + 1].ins, dmas[i].ins, sync=False,
                               reason="DMA issue order")
            ea_sb = aux_sb[:, 0:3 * Q]
            attr_f32v = aux_sb[:, 3 * Q:9 * Q]
            a_sb = aux_sb[:, 9 * Q:9 * Q + 6]

            # ---- weight computation (DVE + one ACT sin); runs in the DVE
            # idle window while the first data chunks stream in.
            # inverse-frequency: ivb[p,j] = sum(anum)/anum[j]
            tot = cpool.tile([P, 1], f32)
            nc.vector.tensor_reduce(tot[:], a_sb, axis=AX.X, op=OP.add)
            rec = cpool.tile([P, 6], f32)
            nc.vector.reciprocal(rec[:], a_sb)
            ivb = cpool.tile([P, 6], f32)
            nc.vector.tensor_scalar_mul(ivb[:], rec[:], tot[:, 0:1])
            # attr_w[p,q] = sum_j attr[j, p*128+q] * iv[j]
            # (attribute is host-converted to f32 0.0/1.0 inside aux)
            aw0 = cpool.tile([P, Q], f32)
            aw1 = cpool.tile([P, Q], f32)
            nc.vector.tensor_scalar_mul(aw0[:], attr_f32v[:, 0:Q],
                                        ivb[:, 0:1])
            cur, nxt = aw0, aw1
            for j in range(1, 6):
                nc.vector.scalar_tensor_tensor(
                    nxt[:], attr_f32v[:, j * Q:(j + 1) * Q], ivb[:, j:j + 1],
                    cur[:], op0=OP.mult, op1=OP.add,
                )
                cur, nxt = nxt, cur
            aw = cur
            # angle_w[p,q] = 2 * sum_j sin(ea[j, p*128+q]/2)^2
            sinh_sb = cpool.tile([P, 3 * Q], f32)
            nc.scalar.activation(sinh_sb[:], ea_sb, AF.Sin, bias=0.0,
                                 scale=0.5)
            ssq = cpool.tile([P, 3 * Q], f32)
            nc.vector.tensor_mul(ssq[:], sinh_sb[:], sinh_sb[:])
            angle = cpool.tile([P, Q], f32)
            nc.vector.tensor_reduce(
                angle[:], ssq[:].rearrange("p (j q) -> p q j", q=Q),
                axis=AX.X, op=OP.add,
            )
            # weight[p,q] = (2*angle) * attr_w
            w_sb = cpool.tile([P, Q], f32)
            nc.vector.scalar_tensor_tensor(
                w_sb[:], angle[:], 2.0, aw[:], op0=OP.mult, op1=OP.mult,
            )
            # ones vector for the final cross-partition matmul reduce
            ones = cpool.tile([P, 1], f32)
            nc.gpsimd.memset(ones[:], 1.0)

            # ---- main loop (all bf16 in place): diff over the label half,
            # square on ACT, per-sample d-reduce into f32 colsq columns.
            colsq = cpool.tile([P, Q], f32)
            subs = []
            reds = []
            q0 = 0
            for c, cb in enumerate(CHUNK_B):
                f = cb * D
                dt_ = dts[c]
                subs.append(nc.vector.tensor_sub(
                    dt_[:, f:2 * f], dt_[:, 0:f], dt_[:, f:2 * f]))
                nc.scalar.activation(dt_[:, f:2 * f], dt_[:, f:2 * f],
                                     AF.Square)
                reds.append(nc.vector.tensor_reduce(
                    colsq[:, q0:q0 + cb],
                    dt_[:, f:2 * f].rearrange("p (b d) -> p b d", d=D),
                    axis=AX.X, op=OP.add,
                ))
                q0 += cb
            # Pin the DVE stream to sub0, sub1, red0, sub2, red1, ... so each
            # chunk's ACT square overlaps the next chunk's subtract (the
            # scheduler's DMA cost model would otherwise serialize them).
            for c in range(NCHUNK - 1):
                add_dep_helper(reds[c].ins, subs[c + 1].ins, sync=False,
                               reason="pipeline: sub(c+1) before red(c)")
                if c + 2 < NCHUNK:
                    add_dep_helper(subs[c + 2].ins, reds[c].ins, sync=False,
                                   reason="pipeline: red(c) before sub(c+2)")

            # ---- partial[p] = sum_q colsq[p,q] * weight[p,q] (one fused
            # DVE op), then reduce across partitions on the TensorEngine and
            # write a single f32.
            scr = cpool.tile([P, Q], f32)
            part = cpool.tile([P, 1], f32)
            nc.vector.scalar_tensor_tensor(
                scr[:], colsq[:], 1.0, w_sb[:], op0=OP.bypass, op1=OP.mult,
                accum_out=part[:],
            )
            ps = ppool.tile([1, 1], f32)
            nc.tensor.matmul(ps[:], ones[:], part[:], start=True, stop=True)
            res = cpool.tile([1, 1], f32)
            nc.vector.tensor_copy(res[:], ps[:])
            nc.sync.dma_start(out.ap(), res[:])

    nc.compile()
    return nc


def _get_program():
    global _program
    if _program is None:
        _program = _build_program()
    return _program


def _make_in_maps(inp, label, ea, attribute, attribute_num):
    inp = np.asarray(inp, dtype=np.float32)
    label = np.asarray(label, dtype=np.float32)
    ea = np.asarray(ea, dtype=np.float32)
    attribute = np.asarray(attribute, dtype=np.int32)
    anum_row = np.asarray(attribute_num, dtype=np.float32).reshape(6)
    in_maps = []
    for c in range(N_CORES):
        s = slice(c * BS, (c + 1) * BS)
        it = np.ascontiguousarray(inp[:, s].T).reshape(P, Q, D)
        lt = np.ascontiguousarray(label[:, s].T).reshape(P, Q, D)
        dat = np.empty((P, TOT_F), dtype=np.float32)
        off = 0
        q0 = 0
        for cb in CHUNK_B:
            f = cb * D
            dat[:, off:off + f] = it[:, q0:q0 + cb].reshape(P, f)
            dat[:, off + f:off + 2 * f] = lt[:, q0:q0 + cb].reshape(P, f)
            off += 2 * f
            q0 += cb
        aux = np.empty((P, AUX_F), dtype=np.float32)
        aux[:, 0:3 * Q] = (
            ea[:, s].reshape(3, P, Q).transpose(1, 0, 2).reshape(P, 3 * Q))
        aux[:, 3 * Q:9 * Q] = (
            attribute[:, s].reshape(6, P, Q).transpose(1, 0, 2)
            .reshape(P, 6 * Q).astype(np.float32))
        aux[:, 9 * Q:9 * Q + 6] = anum_row
        in_maps.append({"data": dat, "aux": aux})
    return in_maps


def run(inputs, trace=False, trace_cores=None):
    """Run on hardware; returns (result_scalar, BassKernelResults)."""
    try:
        from concourse.bass_utils import run_bass_kernel_spmd
    except ImportError:
        sys.path.insert(0, "/opt/trn_rl_repo")
        from concourse.bass_utils import run_bass_kernel_spmd
    nc = _get_program()
    in_maps = _make_in_maps(**inputs)
    kwargs = {}
    if trace:
        kwargs["trace"] = True
        if trace_cores is not None:
            kwargs["trace_cores"] = trace_cores
    res = run_bass_kernel_spmd(nc, in_maps, core_ids=list(range(N_CORES)), **kwargs)
    total = 0.0
    for r in res.results:
        total += float(r["out"].astype(np.float64).sum())
    value = np.asarray(total / (D * B), dtype=np.float32)
    return value, res


def kernel(**inputs):
    value, _ = run(inputs)
    return value
